# revision 2
# baseline (speedup 1.0000x reference)
"""Trainium2 Bass kernel v3: bf16 transposed-stream with TensorE reductions.

Host (layout/dtype only): per core, transposes each input to
[spatial=4096, rows=640], converts to bf16, and packs per spatial position
a 4484-element row: [stat(1,locx%2,locy,blk), m, fd, bd, fvx, fvy, bvx, bvy]
(each slot 640 rows wide). Device streams 32 blocks of 128 spatial
positions: ONE DMA per block, two wide bf16 VectorE multiplies
(t = [fd|bd] * m, U = [fvx fvy bvx bvy] * [tf tf tb tb]), and 7
accumulating TensorE matmuls against the 4-column stat stationary that
reduce over the spatial partitions into PSUM: mask moments
(msum, locx-part, locy, blk-weighted msum) and the 4 vector dot sums.
After 32 blocks the accumulated [4,640]+[4,2560] PSUM stats are copied out.
The tiny keypoint assembly runs on host (folding the res=64 scale and the
locx block-offset correction: locx_global = locx%2 + 2*blk).
"""

import sys

if "/opt/trn_rl_repo" not in sys.path:
    sys.path.insert(0, "/opt/trn_rl_repo")

import numpy as np
import ml_dtypes

import concourse.bass as bass
import concourse.tile as tile
from concourse import mybir
from concourse.bass_utils import run_bass_kernel_spmd

N_CORES = 8
B_FULL = 256
B_SHARD = B_FULL // N_CORES  # 32
C = 20
RES = 64
SPATIAL = RES * RES          # 4096
ROWS = B_SHARD * C           # 640 rows per core
P = 128                      # spatial positions per block (partitions)
NBLK = SPATIAL // P          # 32
NSLOT = 7                    # m, fd, bd, fvx, fvy, bvx, bvy
WROW = 4 + NSLOT * ROWS      # 4484 packed row elements
EPS = 1e-6

F32 = mybir.dt.float32
BF16 = mybir.dt.bfloat16
NP_BF16 = ml_dtypes.bfloat16

# packed row offsets
O_STAT = 0
O_M = 4
O_FD = 4 + 1 * ROWS
O_BD = 4 + 2 * ROWS
O_VEC = 4 + 3 * ROWS         # fvx, fvy, bvx, bvy (4*ROWS)


def _build_program(repeat: int = 1) -> bass.Bass:
    nc = bass.Bass()

    pk = nc.declare_dram_parameter("packed", [NBLK, P, WROW], BF16, isOutput=False)
    sm_o = nc.declare_dram_parameter("stats_m", [4, ROWS], F32, isOutput=True)
    su_o = nc.declare_dram_parameter("stats_u", [1, 4 * ROWS], F32, isOutput=True)

    with tile.TileContext(nc) as tc:
        with (
            tc.tile_pool(name="io", bufs=6) as io,
            tc.tile_pool(name="work", bufs=3) as work,
            tc.tile_pool(name="out", bufs=1) as outp,
            tc.tile_pool(name="psum", bufs=1, space="PSUM") as psum,
        ):
            psum_m = psum.tile([4, ROWS], F32, tag="pm")
            psum_u = psum.tile([4, 4 * ROWS], F32, tag="pu")

            it = 0
            for gb in range(NBLK * repeat):
                b = gb % NBLK
                first = b == 0
                last = b == NBLK - 1
                pk_t = io.tile([P, WROW], BF16, tag="pk")
                eng = nc.sync if (it % 2 == 0) else nc.scalar
                eng.dma_start(out=pk_t, in_=pk[b, :, :])
                it += 1

                st = pk_t[:, O_STAT : O_STAT + 4]
                T = work.tile([P, 2, ROWS], BF16, tag="T")
                fdbd = pk_t[:, O_FD : O_FD + 2 * ROWS].rearrange(
                    "p (s f) -> p s f", s=2
                )
                m2 = pk_t[:, O_M : O_M + ROWS].unsqueeze(1).to_broadcast([P, 2, ROWS])
                nc.vector.tensor_mul(T, fdbd, m2)

                U = work.tile([P, 2, 2, ROWS], BF16, tag="U")
                vecs = pk_t[:, O_VEC : O_VEC + 4 * ROWS].rearrange(
                    "p (a b f) -> p a b f", a=2, b=2
                )
                T4 = T.unsqueeze(2).to_broadcast([P, 2, 2, ROWS])
                nc.vector.tensor_mul(U, vecs, T4)

                nc.tensor.matmul(
                    out=psum_m[:, 0:512],
                    lhsT=st,
                    rhs=pk_t[:, O_M : O_M + 512],
                    start=first,
                    stop=last,
                )
                nc.tensor.matmul(
                    out=psum_m[:, 512:ROWS],
                    lhsT=st,
                    rhs=pk_t[:, O_M + 512 : O_M + ROWS],
                    start=first,
                    stop=last,
                )
                Uf = U.rearrange("p a b f -> p (a b f)")
                for k in range(5):
                    nc.tensor.matmul(
                        out=psum_u[:, k * 512 : (k + 1) * 512],
                        lhsT=st,
                        rhs=Uf[:, k * 512 : (k + 1) * 512],
                        start=first,
                        stop=last,
                    )

            sm = outp.tile([4, ROWS], F32, tag="sm")
            su = outp.tile([1, 4 * ROWS], F32, tag="su")
            nc.scalar.copy(out=sm, in_=psum_m)
            nc.scalar.copy(out=su, in_=psum_u[0:1, :])
            nc.sync.dma_start(out=sm_o[:, :], in_=sm)
            nc.sync.dma_start(out=su_o[:, :], in_=su)

    from concourse.library_overlay import lower_extended_insts

    lower_extended_insts(nc)
    _legalize_waits(nc)
    return nc


def _legalize_waits(nc) -> None:
    """walrus codegen allows 1 sync-wait per instruction (2 for
    EventSemaphore). Hoist excess waits onto EventSemaphore carriers
    inserted just before the offending instruction on the same engine."""
    for f in nc.m.functions:
        for blk in f.blocks:
            insts = blk.instructions
            new_list = []
            changed = False
            for ins in insts:
                si = getattr(ins, "sync_info", None)
                ow = list(si.on_wait) if (si is not None and si.on_wait) else []
                cap = 2 if isinstance(ins, mybir.InstEventSemaphore) else 1
                if len(ow) > cap:
                    excess, keep = ow[:-cap], ow[-cap:]
                    for j in range(0, len(excess), 2):
                        ev = mybir.InstEventSemaphore(
                            name=f"{ins.name}-lw{j}", ins=[], outs=[]
                        )
                        ev.engine = ins.engine
                        ev.sync_info = mybir.SyncInfo(
                            on_wait=excess[j : j + 2], on_update=[]
                        )
                        new_list.append(ev)
                    ins.sync_info = mybir.SyncInfo(
                        on_wait=keep,
                        on_update=list(si.on_update) if si.on_update else [],
                    )
                    changed = True
                new_list.append(ins)
            if changed:
                blk.instructions.clear()
                blk.instructions.extend(new_list)


_PROGRAM_CACHE: dict = {}


def _get_program() -> bass.Bass:
    if "nc" not in _PROGRAM_CACHE:
        _PROGRAM_CACHE["nc"] = _build_program()
    return _PROGRAM_CACHE["nc"]


def _run_device(in_maps, trace=False, **kwargs):
    nc = _get_program()
    return run_bass_kernel_spmd(nc, in_maps, list(range(N_CORES)), trace=trace, **kwargs)


def _make_in_maps(front_vec, front_dis, back_vec, back_dis, ske_mask):
    R = B_FULL * C
    fv = np.asarray(front_vec, dtype=np.float32).reshape(R, SPATIAL, 2)
    fd = np.asarray(front_dis, dtype=np.float32).reshape(R, SPATIAL)
    bv = np.asarray(back_vec, dtype=np.float32).reshape(R, SPATIAL, 2)
    bd = np.asarray(back_dis, dtype=np.float32).reshape(R, SPATIAL)
    m = np.asarray(ske_mask, dtype=np.float32).reshape(R, SPATIAL)

    s = np.arange(SPATIAL)
    stat = np.empty((SPATIAL, 4), dtype=NP_BF16)
    stat[:, 0] = 1.0
    stat[:, 1] = ((s // RES) % 2).astype(NP_BF16)
    stat[:, 2] = (s % RES).astype(NP_BF16)
    stat[:, 3] = (s // P).astype(NP_BF16)

    in_maps = []
    for i in range(N_CORES):
        sl = slice(i * ROWS, (i + 1) * ROWS)
        packed = np.empty((SPATIAL, WROW), dtype=NP_BF16)
        packed[:, 0:4] = stat
        packed[:, O_M : O_M + ROWS] = m[sl].T.astype(NP_BF16)
        packed[:, O_FD : O_FD + ROWS] = fd[sl].T.astype(NP_BF16)
        packed[:, O_BD : O_BD + ROWS] = bd[sl].T.astype(NP_BF16)
        packed[:, O_VEC + 0 * ROWS : O_VEC + 1 * ROWS] = fv[sl, :, 0].T.astype(NP_BF16)
        packed[:, O_VEC + 1 * ROWS : O_VEC + 2 * ROWS] = fv[sl, :, 1].T.astype(NP_BF16)
        packed[:, O_VEC + 2 * ROWS : O_VEC + 3 * ROWS] = bv[sl, :, 0].T.astype(NP_BF16)
        packed[:, O_VEC + 3 * ROWS : O_VEC + 4 * ROWS] = bv[sl, :, 1].T.astype(NP_BF16)
        in_maps.append({"packed": packed.reshape(NBLK, P, WROW)})
    return in_maps


def _assemble(sm_all: np.ndarray, su_all: np.ndarray) -> np.ndarray:
    """sm_all: [ncore, 4, ROWS]; su_all: [ncore, 4*ROWS] -> kp [B, 21, 2]."""
    B = B_FULL
    msum = sm_all[:, 0, :].reshape(B, C).astype(np.float32)
    mxl = sm_all[:, 1, :].reshape(B, C).astype(np.float32)
    my = sm_all[:, 2, :].reshape(B, C).astype(np.float32)
    mb = sm_all[:, 3, :].reshape(B, C).astype(np.float32)
    u = su_all.reshape(N_CORES, 4, ROWS)
    ufx = u[:, 0, :].reshape(B, C).astype(np.float32)
    ufy = u[:, 1, :].reshape(B, C).astype(np.float32)
    ubx = u[:, 2, :].reshape(B, C).astype(np.float32)
    uby = u[:, 3, :].reshape(B, C).astype(np.float32)

    mx = mxl + np.float32(2.0) * mb
    r = np.float32(1.0) / (msum + np.float32(EPS))
    s64 = np.float32(RES)
    F_ = np.stack([(s64 * ufx + mx) * r, (s64 * ufy + my) * r], -1)
    Bk = np.stack([(s64 * ubx + mx) * r, (s64 * uby + my) * r], -1)

    root_terms = np.where(
        (msum[:, ::4] != 0.0)[..., None], Bk[:, ::4], np.float32(0.0)
    )  # [B,5,2]
    kp0 = root_terms.sum(axis=1, dtype=np.float32) / np.float32(5.0)  # [B,2]

    Fg = F_.reshape(B, 5, 4, 2)
    Bg = Bk.reshape(B, 5, 4, 2)
    tail = np.stack(
        [
            Fg[:, :, 3],
            (Fg[:, :, 2] + Bg[:, :, 3]) * np.float32(0.5),
            (Fg[:, :, 1] + Bg[:, :, 2]) * np.float32(0.5),
            (Fg[:, :, 0] + Bg[:, :, 1]) * np.float32(0.5),
        ],
        axis=2,
    )  # [B,5,4,2]
    kp = np.concatenate([kp0[:, None], tail.reshape(B, 20, 2)], axis=1)
    return (kp * np.float32(4.0)).astype(np.float32)


def kernel(front_vec, front_dis, back_vec, back_dis, ske_mask) -> np.ndarray:
    in_maps = _make_in_maps(front_vec, front_dis, back_vec, back_dis, ske_mask)
    res = _run_device(in_maps)
    sm_all = np.stack([np.asarray(res.results[i]["stats_m"]) for i in range(N_CORES)])
    su_all = np.stack(
        [np.asarray(res.results[i]["stats_u"])[0] for i in range(N_CORES)]
    )
    return _assemble(sm_all, su_all)
